# revision 37
# baseline (speedup 1.0000x reference)
"""AdaPT int8-quantized 3x3 conv (systolic, exact) on 8 TRN2 NeuronCores.

Full inputs: x [32,8,384,384] f32, weight [8,8,3,3] f32, bias [8] f32.
Sharding: data-parallel over batch (4 images per core), amax all-reduced
(max) across cores, weights/bias replicated.

Per-core plan:
  - load x in two 192-row halves into a wide [128, 49, 384] SBUF layout
    (partition = (q, img, ci), q = 49-row block), junk rows zeroed
  - DVE abs-max reduce + gpsimd partition all-reduce -> local amax;
    AllReduce(max) over the 8 cores via a DRAM bounce buffer
  - quantize wide with the fp32 magic-number round (bit-exact RNE, same
    as jnp.round), output bf16 (ints <= 127 are exact in bf16)
  - SBUF->SBUF DMA builds a dx-shifted x3-replicated rhs chunk
    [128, 34, 384] (partition = 32*img + ci*3 + dx)
  - conv: per 512-wide PSUM bank, 4 column-band matmul tiles
    (tile_position (32c, 32c), one image per band, concurrent) x 3
    accumulating dy-matmuls with row-shifted rhs windows; weights are
    bf16 slices of a per-band-replicated stationary tensor
  - evacuate PSUM [128,512] full-width: out = psum * (1/(sx*sw)) + bias
  - DMA the 32 useful partitions (4 img x 8 co) to HBM

All long-lived SBUF buffers use static allocations (alloc_sbuf_tensor):
the tile-pool allocator reuses slots by inferred lifetime and was
observed overlapping long-lived tiles.
"""

import numpy as np

N_CORES = 8
IMG = 4          # images per core
CI = 8
CO = 8
H = W = 384
HALF = 192       # rows per half
QROWS = 49       # rows per partition block (4 blocks cover HALF + halo)
CHUNK = 32       # output rows per rhs chunk
NCHUNK = HALF // CHUNK   # 6
NBANK = CHUNK * W // 512  # 24 psum banks (512 cols) per chunk
MAGIC = 12582912.0  # 1.5 * 2**23, fp32 round-to-nearest-int trick
MAX_Q = 127.0

_cached = {}


def _build(n_cores=N_CORES, debug=False):
    from concourse import bacc, bass, tile, mybir, bass_isa

    f32 = mybir.dt.float32
    bf16 = mybir.dt.bfloat16

    nc = bacc.Bacc(
        "TRN2", target_bir_lowering=False, debug=debug, num_devices=n_cores
    )

    x_ext = nc.declare_dram_parameter("x", [IMG, CI, H, W], f32, isOutput=False)
    w_ext = nc.declare_dram_parameter("weight", [CO, CI, 3, 3], f32, isOutput=False)
    b_ext = nc.declare_dram_parameter("bias", [CO], f32, isOutput=False)
    out_ext = nc.declare_dram_parameter("out", [IMG, CO, H, W], f32, isOutput=True)
    dbg_ext = nc.declare_dram_parameter("dbg", [128, 8], f32, isOutput=True)

    # ---- static SBUF buffers (long-lived) ----
    sb = lambda name, shape, dt: nc.alloc_sbuf_tensor(name, list(shape), dt).ap()
    xh = sb("xh_s", [128, QROWS, W], f32)
    qxh = sb("qxh_s", [128, QROWS, W], bf16)
    rep_bufs = [sb(f"rep{t}_s", [128, CHUNK + 2, W], bf16) for t in range(2)]
    w24 = sb("w24_s", [24, 3 * CO], f32)
    qw = sb("qw_s", [128, 48], bf16)
    aw = sb("aw_s", [24, 1], f32)
    aw_all = sb("awall_s", [24, 1], f32)
    sw = sb("sw_s", [24, 1], f32)
    bias_e = sb("biase_s", [128, 1], f32)
    ax = sb("ax_s", [128, 1], f32)
    ax_t = sb("axt_s", [128, 1], f32)
    ax_all = sb("axall_s", [128, 1], f32)
    axg = sb("axg_s", [128, 1], f32)
    sx = sb("sx_s", [128, 1], f32)
    aw128 = sb("aw128_s", [128, 1], f32)
    inv = sb("inv_s", [128, 1], f32)

    with tile.TileContext(nc) as tc:
        with (
            tc.tile_pool(name="stage", bufs=2) as spool,
            tc.tile_pool(name="psum", bufs=6, space="PSUM") as pspool,
            tc.tile_pool(name="dram", bufs=1, space="DRAM") as dpool,
        ):
            # ---------------- weight prep ----------------
            # qw layout (per 32-partition band c, replicated):
            #   qw[32c + kx*8 + ci, 0:8]    = dy0 weights, cols 8:32 zero
            #   qw[32c + kx*8 + ci, 32:40]  = dy1, [40:48] = dy2
            # (dx-major so all DMAs touch contiguous partition ranges)
            with nc.allow_non_contiguous_dma(reason="one-time 576-elem w load"):
                for ky in range(3):
                    for kx in range(3):
                        dst = w24[8 * kx:8 * kx + 8, CO * ky:CO * ky + CO]
                        src = w_ext[:, :, ky, kx].rearrange("co ci -> ci co")
                        nc.sync.dma_start(out=dst, in_=src)

            nc.vector.tensor_reduce(
                aw[:, :], w24[:, :], mybir.AxisListType.X, mybir.AluOpType.max,
                apply_absolute_value=True,
            )
            nc.gpsimd.partition_all_reduce(
                aw_all[:, :], aw[:, :], channels=24, reduce_op=bass_isa.ReduceOp.max
            )
            nc.vector.reciprocal(sw[:, :], aw_all[:, :])
            nc.vector.tensor_scalar(
                out=sw[:, :], in0=sw[:, :], scalar1=MAX_Q, scalar2=None,
                op0=mybir.AluOpType.mult,
            )
            # quantize weights: round(w * sw) via magic, to bf16
            nc.vector.tensor_scalar(
                out=w24[:, :], in0=w24[:, :], scalar1=sw[:, :], scalar2=MAGIC,
                op0=mybir.AluOpType.mult, op1=mybir.AluOpType.add,
            )
            nc.vector.memset(qw[:, :], 0.0)
            nc.scalar.activation(
                qw[0:24, 0:CO], w24[:, 0:CO],
                mybir.ActivationFunctionType.Copy, bias=-MAGIC, scale=1.0,
            )
            nc.scalar.activation(
                qw[0:24, 32:32 + 2 * CO], w24[:, CO:3 * CO],
                mybir.ActivationFunctionType.Copy, bias=-MAGIC, scale=1.0,
            )
            for c in range(1, IMG):
                nc.sync.dma_start(out=qw[32 * c:32 * c + 24, :], in_=qw[0:24, :])

            # bias vector on evac partitions: p = 32*img + co
            nc.vector.memset(bias_e[:, :], 0.0)
            for c in range(IMG):
                nc.sync.dma_start(out=bias_e[32 * c:32 * c + CO, :], in_=b_ext[:])

            # rhs double buffers, zeroed once: pad partitions and the dx
            # edge columns stay zero forever
            for rb in rep_bufs:
                nc.vector.memset(rb[:, :, :], 0.0)

            # ---------------- x amax pass ----------------
            for h in range(2):
                _load_half(nc, xh, x_ext, h)
                nc.vector.tensor_reduce(
                    ax_t[:, :], xh[:, :, :], mybir.AxisListType.XY,
                    mybir.AluOpType.max, apply_absolute_value=True,
                )
                if h == 0:
                    nc.vector.tensor_copy(ax[:, :], ax_t[:, :])
                else:
                    nc.vector.tensor_tensor(
                        out=ax[:, :], in0=ax[:, :], in1=ax_t[:, :],
                        op=mybir.AluOpType.max,
                    )
            nc.gpsimd.partition_all_reduce(
                ax_all[:, :], ax[:, :], channels=128, reduce_op=bass_isa.ReduceOp.max
            )

            # ---------------- amax all-reduce across cores ----------------
            cc_in = dpool.tile([1, 128], f32)
            cc_out = dpool.tile([1, 128], f32)
            nc.sync.dma_start(
                out=cc_in.rearrange("one p -> p one"), in_=ax_all[:, :]
            )
            nc.gpsimd.collective_compute(
                "AllReduce",
                mybir.AluOpType.max,
                replica_groups=[list(range(n_cores))],
                ins=[cc_in.opt()],
                outs=[cc_out.opt()],
            )
            nc.sync.dma_start(
                out=axg[:, :], in_=cc_out.rearrange("one p -> p one")
            )

            # sx = 127/axg  (per-partition, all equal)
            nc.vector.reciprocal(sx[:, :], axg[:, :])
            nc.vector.tensor_scalar(
                out=sx[:, :], in0=sx[:, :], scalar1=MAX_Q, scalar2=None,
                op0=mybir.AluOpType.mult,
            )
            # inv = axg * aw / 127^2
            nc.gpsimd.partition_broadcast(aw128[:, :], aw_all[0:1, :])
            nc.vector.tensor_tensor(
                out=inv[:, :], in0=axg[:, :], in1=aw128[:, :],
                op=mybir.AluOpType.mult,
            )
            nc.vector.tensor_scalar(
                out=inv[:, :], in0=inv[:, :], scalar1=1.0 / (MAX_Q * MAX_Q),
                scalar2=None, op0=mybir.AluOpType.mult,
            )

            # debug: scale-chain intermediates
            dbg = nc.alloc_sbuf_tensor("dbg_s", [128, 8], f32).ap()
            nc.vector.memset(dbg[:, :], 0.0)
            nc.vector.tensor_copy(dbg[:, 0:1], ax[:, :])
            nc.vector.tensor_copy(dbg[:, 1:2], ax_all[:, :])
            nc.vector.tensor_copy(dbg[:, 2:3], axg[:, :])
            nc.vector.tensor_copy(dbg[:, 3:4], sx[:, :])
            nc.vector.tensor_copy(dbg[:, 4:5], inv[:, :])
            nc.vector.tensor_copy(dbg[:, 5:6], aw128[:, :])
            nc.vector.tensor_copy(dbg[0:24, 6:7], aw_all[:, :])
            nc.vector.tensor_copy(dbg[0:24, 7:8], sw[:, :])
            nc.sync.dma_start(out=dbg_ext[:, :], in_=dbg[:, :])

            # ---------------- main loop over halves ----------------
            out_flat = out_ext.rearrange("i co h w -> i co (h w)")
            for h in (1, 0):
                if h == 0:
                    _load_half(nc, xh, x_ext, 0)
                # quantize wide: round(x*sx) -> bf16
                nc.vector.tensor_scalar(
                    out=xh[:, :, :], in0=xh[:, :, :], scalar1=sx[:, :],
                    scalar2=MAGIC, op0=mybir.AluOpType.mult,
                    op1=mybir.AluOpType.add,
                )
                nc.scalar.activation(
                    qxh[:, :, :], xh[:, :, :], mybir.ActivationFunctionType.Copy,
                    bias=-MAGIC, scale=1.0,
                )

                for r in range(NCHUNK):
                    # rhs chunk: rep[32*img + dx*8 + ci, rr, xo]
                    #   = qx[img, ci, h*192 + 32*r - 1 + rr, xo + dx - 1]
                    rep = rep_bufs[((1 - h) * NCHUNK + r) % 2]
                    b0 = CHUNK * r  # buffer row of chunk start (y - 1)
                    pieces = []
                    bb = b0
                    while bb < b0 + CHUNK + 2:
                        q = bb // QROWS
                        n = min((q + 1) * QROWS, b0 + CHUNK + 2) - bb
                        pieces.append((q, bb - q * QROWS, bb - b0, n))
                        bb += n
                    rep_eng = nc.sync if r % 2 == 0 else nc.gpsimd
                    for i in range(IMG):
                        for dx in range(3):
                            xs, xe = max(0, 1 - dx), W - max(0, dx - 1)
                            for (q, qr, rr, n) in pieces:
                                p0 = 32 * q + 8 * i
                                d0 = 32 * i + 8 * dx
                                rep_eng.dma_start(
                                    out=rep[d0:d0 + 8, rr:rr + n, xs:xe],
                                    in_=qxh[p0:p0 + 8, qr:qr + n,
                                            xs + dx - 1:xe + dx - 1],
                                )

                    rep_f = rep.rearrange("p r x -> p (r x)")
                    st = None
                    for wb in range(NBANK):
                        ps = pspool.tile([128, 512], f32, tag="ps")
                        for dy in range(3):
                            for c in range(IMG):
                                off = dy * W + wb * 512
                                if dy == 0:
                                    # M=32: cols 8:32 are zero weights so
                                    # pad psum partitions get written zeros
                                    lhsT = qw[32 * c:32 * c + 24, 0:32]
                                    out_ap = ps[32 * c:32 * c + 32, :]
                                else:
                                    lhsT = qw[32 * c:32 * c + 24,
                                              24 + CO * dy:24 + CO * dy + CO]
                                    out_ap = ps[32 * c:32 * c + CO, :]
                                nc.tensor.matmul(
                                    out_ap,
                                    lhsT,
                                    rep_f[32 * c:32 * c + 24, off:off + 512],
                                    start=(dy == 0),
                                    stop=(dy == 2),
                                    skip_group_check=True,
                                    tile_position=(32 * c, 32 * c),
                                )
                        # batch 8 psum banks into one staging tile; one
                        # output DMA per image per group, on the ACT queue
                        g = wb % 8
                        if g == 0:
                            st = spool.tile([128, 8 * 512], f32, tag="st")
                        if wb % 2 == 0:
                            nc.vector.tensor_scalar(
                                out=st[:, 512 * g:512 * g + 512], in0=ps[:, :],
                                scalar1=inv[:, :], scalar2=bias_e[:, :],
                                op0=mybir.AluOpType.mult,
                                op1=mybir.AluOpType.add,
                            )
                        else:
                            # out = Identity(psum * inv + bias) on ScalarE
                            nc.scalar.activation(
                                st[:, 512 * g:512 * g + 512], ps[:, :],
                                mybir.ActivationFunctionType.Identity,
                                bias=bias_e[:, :], scale=inv[:, :],
                            )
                        if g == 7:
                            off_out = ((h * HALF + CHUNK * r) * W
                                       + (wb - 7) * 512)
                            for i in range(IMG):
                                nc.scalar.dma_start(
                                    out=out_flat[i, :, off_out:off_out + 4096],
                                    in_=st[32 * i:32 * i + CO, :],
                                )

    nc.compile()
    return nc


def _load_half(nc, xh, x_ext, h):
    """Load rows so that xh[q*32 + i*8 + c, rr, :] = x[i, c, y, :] with
    y = h*192 - 1 + q*49 + rr.  Junk rows (y < 0 or y > 383) zeroed.
    One DMA covers all 4 images (32 contiguous partitions); rows are
    split into <=25-row pieces to keep AP dims under the 16k-elem cap."""
    def load_q(q, r0, nrows, y0):
        r = 0
        while r < nrows:
            n = min(25, nrows - r)
            nc.sync.dma_start(
                out=xh[32 * q:32 * q + 32, r0 + r:r0 + r + n, :],
                in_=x_ext[:, :, y0 + r:y0 + r + n, :],
            )
            r += n

    if h == 0:
        nc.vector.memset(xh[0:32, 0:1, :], 0.0)
        load_q(0, 1, 48, 0)
        for q in range(1, 4):
            load_q(q, 0, 49, q * 49 - 1)
    else:
        for q in range(3):
            load_q(q, 0, 49, 191 + q * 49)
        load_q(3, 0, 46, 338)
        nc.vector.memset(xh[96:128, 46:49, :], 0.0)


def _get_nc():
    if "nc" not in _cached:
        _cached["nc"] = _build()
    return _cached["nc"]


def kernel(x, weight, bias):
    from concourse.bass_utils import run_bass_kernel_spmd

    nc = _get_nc()
    in_maps = [
        {
            "x": np.ascontiguousarray(x[i * IMG:(i + 1) * IMG], dtype=np.float32),
            "weight": np.ascontiguousarray(weight, dtype=np.float32),
            "bias": np.ascontiguousarray(bias, dtype=np.float32),
        }
        for i in range(N_CORES)
    ]
    res = run_bass_kernel_spmd(nc, in_maps, core_ids=list(range(N_CORES)))
    out = np.concatenate([res.results[i]["out"] for i in range(N_CORES)], axis=0)
    return out.astype(np.float32)
